# revision 4
# baseline (speedup 1.0000x reference)
import sys

for _p in ("/opt/trn_rl_repo", "/opt/trn_rl_repo/concourse"):
    if _p not in sys.path:
        sys.path.insert(0, _p)

import numpy as np
import concourse.bass as bass
import concourse.bacc as bacc
import concourse.mybir as mybir
import concourse.tile as tile

P = 128
D = 512
S = 1600
K = 64
NIMG = 4          # images per core
NCORES = 8
NCHUNK = 13       # 12*128 + 64 = 1600
FR = mybir.dt.float32r
F32 = mybir.dt.float32
AF = mybir.ActivationFunctionType
OP = mybir.AluOpType


def build():
    nc = bacc.Bacc("TRN2", target_bir_lowering=False, debug=False,
                   enable_asserts=True, num_devices=NCORES)
    X_d = nc.dram_tensor("X", [NIMG, 4, P, S], F32, kind="ExternalInput").ap()
    WT_d = nc.dram_tensor("WT", [4, P, K], F32, kind="ExternalInput").ap()
    EYE_d = nc.dram_tensor("EYE", [P, P], F32, kind="ExternalInput").ap()
    CENT_d = nc.dram_tensor("CENT", [K, D], F32, kind="ExternalInput").ap()
    OUT_d = nc.dram_tensor("OUT", [NIMG, K, D], F32, kind="ExternalOutput").ap()

    with tile.TileContext(nc) as tc:
        with tc.tile_pool(name="const", bufs=1) as cpool, \
             tc.tile_pool(name="xn", bufs=2) as xpool, \
             tc.tile_pool(name="xts", bufs=3) as xtpool, \
             tc.tile_pool(name="exp", bufs=3) as epool, \
             tc.tile_pool(name="cols", bufs=3) as colpool, \
             tc.tile_pool(name="sqscr", bufs=1) as sqpool, \
             tc.tile_pool(name="fc", bufs=2) as fcpool, \
             tc.tile_pool(name="nv", bufs=2) as nvpool, \
             tc.tile_pool(name="sq64", bufs=2) as sq64pool, \
             tc.tile_pool(name="outp", bufs=2) as outpool, \
             tc.tile_pool(name="ps_xT", bufs=2, space=bass.MemorySpace.PSUM) as ps_xT, \
             tc.tile_pool(name="ps_lg", bufs=2, space=bass.MemorySpace.PSUM) as ps_lg, \
             tc.tile_pool(name="ps_A", bufs=2, space=bass.MemorySpace.PSUM) as ps_A, \
             tc.tile_pool(name="ps_B", bufs=2, space=bass.MemorySpace.PSUM) as ps_B:

            wt = cpool.tile([P, 4, K], FR)
            ident = cpool.tile([P, P], FR)
            cent = cpool.tile([K, D], F32)
            for g in range(4):
                nc.sync.dma_start(wt[:, g, :], WT_d[g, :, :].bitcast(FR))
            nc.sync.dma_start(ident[:], EYE_d[:, :].bitcast(FR))
            nc.sync.dma_start(cent[:], CENT_d[:, :])
            sqscr = sqpool.tile([P, D], F32)

            for n in range(NIMG):
                xn = xpool.tile([P, 4, S], FR)
                for g in range(4):
                    nc.sync.dma_start(xn[:, g, :], X_d[n, g, :, :].bitcast(FR))
                aggA = ps_A.tile([K, 256], F32)
                aggB = ps_B.tile([K, 258], F32)
                for j in range(NCHUNK):
                    w = 64 if j == NCHUNK - 1 else 128
                    s0 = j * 128
                    xTp = ps_xT.tile([P, D], F32)
                    lgp = ps_lg.tile([P, K], F32)
                    cols = colpool.tile([P, 8], F32)
                    for g in range(4):
                        nc.tensor.transpose(
                            xTp[0:w, g * 128:(g + 1) * 128].bitcast(FR),
                            xn[:, g, s0:s0 + w], ident[:, :])
                    for g in range(4):
                        nc.tensor.matmul(lgp[0:w, :], xn[:, g, s0:s0 + w],
                                         wt[:, g, :], start=(g == 0), stop=(g == 3))
                    # ss = sum_d x^2 per pixel; inv = 1/sqrt(ss)
                    nc.scalar.activation(out=sqscr[0:w, :], in_=xTp[0:w, :],
                                         func=AF.Square, accum_out=cols[0:w, 0:1])
                    nc.scalar.activation(out=cols[0:w, 1:2], in_=cols[0:w, 0:1],
                                         func=AF.Sqrt)
                    nc.vector.reciprocal(cols[0:w, 2:3], cols[0:w, 1:2])
                    # e = exp(logits * inv); sumexp
                    expt = epool.tile([P, K], FR)
                    nc.scalar.activation(out=expt[0:w, :], in_=lgp[0:w, :],
                                         func=AF.Exp, scale=cols[0:w, 2:3],
                                         accum_out=cols[0:w, 3:4])
                    nc.vector.reciprocal(cols[0:w, 4:5], cols[0:w, 3:4])  # rse
                    nc.vector.tensor_scalar(out=cols[0:w, 5:6], in0=cols[0:w, 2:3],
                                            scalar1=cols[0:w, 4:5], scalar2=None,
                                            op0=OP.mult)  # combo = inv*rse
                    xts = xtpool.tile([P, 514], FR)
                    # col 512 = rse (gives a_sum); col 513 = combo (dummy, unused)
                    nc.vector.tensor_scalar(out=xts[0:w, 512:514], in0=cols[0:w, 4:6],
                                            scalar1=1.0, scalar2=None, op0=OP.mult)
                    nc.vector.tensor_scalar(out=xts[0:w, 0:512], in0=xTp[0:w, :],
                                            scalar1=cols[0:w, 5:6], scalar2=None,
                                            op0=OP.mult)
                    nc.tensor.matmul(aggA[:, :], expt[0:w, :], xts[0:w, 0:256],
                                     start=(j == 0), stop=(j == NCHUNK - 1))
                    nc.tensor.matmul(aggB[:, :], expt[0:w, :], xts[0:w, 256:514],
                                     start=(j == 0), stop=(j == NCHUNK - 1))
                # finale: nv = asum*cent - agg = -vlad; out = nv * (-1/(8*||vlad_k||))
                fc = fcpool.tile([K, 8], F32)
                nv = nvpool.tile([K, D], F32)
                sq64 = sq64pool.tile([K, D], F32)
                nc.scalar.copy(fc[:, 0:1], aggB[:, 256:257])  # a_sum
                nc.vector.scalar_tensor_tensor(out=nv[:, 0:256], in0=cent[:, 0:256],
                                               scalar=fc[:, 0:1], in1=aggA[:, :],
                                               op0=OP.mult, op1=OP.subtract)
                nc.vector.scalar_tensor_tensor(out=nv[:, 256:512], in0=cent[:, 256:512],
                                               scalar=fc[:, 0:1], in1=aggB[:, 0:256],
                                               op0=OP.mult, op1=OP.subtract)
                nc.scalar.activation(out=sq64[:, :], in_=nv[:, :], func=AF.Square,
                                     accum_out=fc[:, 1:2])
                nc.scalar.activation(out=fc[:, 2:3], in_=fc[:, 1:2], func=AF.Sqrt,
                                     scale=64.0)  # 8*sqrt(rowss)
                nc.vector.reciprocal(fc[:, 3:4], fc[:, 2:3])
                nc.vector.tensor_scalar(out=fc[:, 4:5], in0=fc[:, 3:4],
                                        scalar1=-1.0, scalar2=None, op0=OP.mult)
                ot = outpool.tile([K, D], F32)
                nc.vector.tensor_scalar(out=ot[:, :], in0=nv[:, :],
                                        scalar1=fc[:, 4:5], scalar2=None, op0=OP.mult)
                nc.sync.dma_start(OUT_d[n, :, :], ot[:, :])
    nc.compile()
    return nc


_NC = None


def _get_nc():
    global _NC
    if _NC is None:
        _NC = build()
    return _NC


def _prep(x, conv_weight, centroids):
    x = np.ascontiguousarray(np.asarray(x), dtype=np.float32)
    w = np.ascontiguousarray(np.asarray(conv_weight), dtype=np.float32)
    c = np.ascontiguousarray(np.asarray(centroids), dtype=np.float32)
    xs = x.reshape(32, 4, P, S)
    wT = np.ascontiguousarray(w.T).reshape(4, P, K)
    eye = np.eye(P, dtype=np.float32)
    in_maps = [{"X": np.ascontiguousarray(xs[NIMG * i:NIMG * (i + 1)]),
                "WT": wT, "EYE": eye, "CENT": c} for i in range(NCORES)]
    return in_maps


def _run(x, conv_weight, centroids, trace=False):
    from concourse import bass_utils
    nc = _get_nc()
    in_maps = _prep(x, conv_weight, centroids)
    res = bass_utils.run_bass_kernel_spmd(nc, in_maps,
                                          core_ids=list(range(NCORES)),
                                          trace=trace)
    out = np.concatenate(
        [np.asarray(res.results[i]["OUT"]).reshape(NIMG, K * D)
         for i in range(NCORES)], axis=0)
    return out, getattr(res, "exec_time_ns", None)


def kernel(x, conv_weight, centroids):
    out, _ = _run(x, conv_weight, centroids, trace=False)
    return out


# revision 8
# speedup vs baseline: 23163.2012x; 23163.2012x over previous
import sys

for _p in ("/opt/trn_rl_repo", "/opt/trn_rl_repo/concourse"):
    if _p not in sys.path:
        sys.path.insert(0, _p)

import numpy as np
import concourse.bass as bass
import concourse.bacc as bacc
import concourse.mybir as mybir
import concourse.tile as tile

P = 128
D = 512
S = 1600
K = 64
NIMG = 4          # images per core
NCORES = 8
NCHUNK = 13       # 12*128 + 64 = 1600
FR = mybir.dt.float32r
F32 = mybir.dt.float32
AF = mybir.ActivationFunctionType
OP = mybir.AluOpType


def build():
    nc = bacc.Bacc("TRN2", target_bir_lowering=False, debug=False,
                   enable_asserts=True, num_devices=NCORES)
    X_d = nc.dram_tensor("X", [NIMG, 4, P, S], F32, kind="ExternalInput").ap()
    WT_d = nc.dram_tensor("WT", [4, P, K], F32, kind="ExternalInput").ap()
    EYE_d = nc.dram_tensor("EYE", [P, P], F32, kind="ExternalInput").ap()
    CENT_d = nc.dram_tensor("CENT", [K, D], F32, kind="ExternalInput").ap()
    OUT_d = nc.dram_tensor("OUT", [NIMG, K, D], F32, kind="ExternalOutput").ap()

    with tile.TileContext(nc) as tc:
        with tc.tile_pool(name="const", bufs=1) as cpool, \
             tc.tile_pool(name="xn", bufs=2) as xpool, \
             tc.tile_pool(name="xts", bufs=3) as xtpool, \
             tc.tile_pool(name="exp", bufs=3) as epool, \
             tc.tile_pool(name="cols", bufs=3) as colpool, \
             tc.tile_pool(name="sqscr", bufs=1) as sqpool, \
             tc.tile_pool(name="fc", bufs=2) as fcpool, \
             tc.tile_pool(name="nv", bufs=2) as nvpool, \
             tc.tile_pool(name="sq64", bufs=2) as sq64pool, \
             tc.tile_pool(name="outp", bufs=2) as outpool, \
             tc.tile_pool(name="ps_xT", bufs=2, space=bass.MemorySpace.PSUM) as ps_xT, \
             tc.tile_pool(name="ps_lg", bufs=2, space=bass.MemorySpace.PSUM) as ps_lg, \
             tc.tile_pool(name="ps_A", bufs=2, space=bass.MemorySpace.PSUM) as ps_A, \
             tc.tile_pool(name="ps_B", bufs=2, space=bass.MemorySpace.PSUM) as ps_B:

            # Preload act-func table set 6 (natural_log_exp_and_others):
            # contains Square+Ln+Exp+Copy, so the fixpoint table-load pass
            # sees every activation satisfied and inserts no per-chunk reloads.
            nc.scalar.add_instruction(mybir.InstLoadActFuncSet(act_func_set_id=6))
            wt = cpool.tile([P, 4, K], FR)
            ident = cpool.tile([P, P], FR)
            cent = cpool.tile([K, D], F32)
            for g in range(4):
                nc.sync.dma_start(wt[:, g, :], WT_d[g, :, :].bitcast(FR))
            nc.sync.dma_start(ident[:], EYE_d[:, :].bitcast(FR))
            nc.sync.dma_start(cent[:], CENT_d[:, :])
            sqscr = sqpool.tile([P, D], F32)

            for n in range(NIMG):
                xn = xpool.tile([P, 4, S], FR)
                for g in range(4):
                    nc.sync.dma_start(xn[:, g, :], X_d[n, g, :, :].bitcast(FR))
                aggA = ps_A.tile([K, 256], F32)
                aggB = ps_B.tile([K, 258], F32)
                for j in range(NCHUNK):
                    w = 64 if j == NCHUNK - 1 else 128
                    s0 = j * 128
                    xTp = ps_xT.tile([P, D], F32)
                    lgp = ps_lg.tile([P, K], F32)
                    cols = colpool.tile([P, 8], F32)
                    for g in range(4):
                        nc.tensor.transpose(
                            xTp[0:w, g * 128:(g + 1) * 128].bitcast(FR),
                            xn[:, g, s0:s0 + w], ident[:, :])
                    for g in range(4):
                        nc.tensor.matmul(lgp[0:w, :], xn[:, g, s0:s0 + w],
                                         wt[:, g, :], start=(g == 0), stop=(g == 3))
                    # ss = sum_d x^2 per pixel; inv = exp(-0.5*ln(ss)) = 1/sqrt(ss)
                    nc.scalar.activation(out=sqscr[0:w, :], in_=xTp[0:w, :],
                                         func=AF.Square, accum_out=cols[0:w, 0:1])
                    nc.scalar.activation(out=cols[0:w, 1:2], in_=cols[0:w, 0:1],
                                         func=AF.Ln)
                    nc.scalar.activation(out=cols[0:w, 2:3], in_=cols[0:w, 1:2],
                                         func=AF.Exp, scale=-0.5)
                    # e = exp(logits * inv); sumexp
                    expt = epool.tile([P, K], FR)
                    nc.scalar.activation(out=expt[0:w, :], in_=lgp[0:w, :],
                                         func=AF.Exp, scale=cols[0:w, 2:3],
                                         accum_out=cols[0:w, 3:4])
                    nc.vector.reciprocal(cols[0:w, 4:5], cols[0:w, 3:4])  # rse
                    nc.vector.tensor_scalar(out=cols[0:w, 5:6], in0=cols[0:w, 2:3],
                                            scalar1=cols[0:w, 4:5], scalar2=None,
                                            op0=OP.mult)  # combo = inv*rse
                    xts = xtpool.tile([P, 514], FR)
                    # col 512 = rse (gives a_sum); col 513 = combo (dummy, unused)
                    nc.vector.tensor_scalar(out=xts[0:w, 512:514], in0=cols[0:w, 4:6],
                                            scalar1=1.0, scalar2=None, op0=OP.mult)
                    nc.vector.tensor_scalar(out=xts[0:w, 0:512], in0=xTp[0:w, :],
                                            scalar1=cols[0:w, 5:6], scalar2=None,
                                            op0=OP.mult)
                    nc.tensor.matmul(aggA[:, :], expt[0:w, :], xts[0:w, 0:256],
                                     start=(j == 0), stop=(j == NCHUNK - 1))
                    nc.tensor.matmul(aggB[:, :], expt[0:w, :], xts[0:w, 256:514],
                                     start=(j == 0), stop=(j == NCHUNK - 1))
                # finale: nv = asum*cent - agg = -vlad; out = nv * (-1/(8*||vlad_k||))
                fc = fcpool.tile([K, 8], F32)
                nv = nvpool.tile([K, D], F32)
                sq64 = sq64pool.tile([K, D], F32)
                nc.scalar.copy(fc[:, 0:1], aggB[:, 256:257])  # a_sum
                nc.vector.scalar_tensor_tensor(out=nv[:, 0:256], in0=cent[:, 0:256],
                                               scalar=fc[:, 0:1], in1=aggA[:, :],
                                               op0=OP.mult, op1=OP.subtract)
                nc.vector.scalar_tensor_tensor(out=nv[:, 256:512], in0=cent[:, 256:512],
                                               scalar=fc[:, 0:1], in1=aggB[:, 0:256],
                                               op0=OP.mult, op1=OP.subtract)
                nc.scalar.activation(out=sq64[:, :], in_=nv[:, :], func=AF.Square,
                                     accum_out=fc[:, 1:2])
                nc.scalar.activation(out=fc[:, 2:3], in_=fc[:, 1:2], func=AF.Ln)
                nc.scalar.activation(out=fc[:, 3:4], in_=fc[:, 2:3], func=AF.Exp,
                                     scale=-0.5)  # 1/sqrt(rowss)
                nc.vector.tensor_scalar(out=fc[:, 4:5], in0=fc[:, 3:4],
                                        scalar1=-0.125, scalar2=None, op0=OP.mult)
                ot = outpool.tile([K, D], F32)
                nc.vector.tensor_scalar(out=ot[:, :], in0=nv[:, :],
                                        scalar1=fc[:, 4:5], scalar2=None, op0=OP.mult)
                nc.sync.dma_start(OUT_d[n, :, :], ot[:, :])
    nc.compile()
    return nc


_NC = None


def _get_nc():
    global _NC
    if _NC is None:
        _NC = build()
    return _NC


def _prep(x, conv_weight, centroids):
    x = np.ascontiguousarray(np.asarray(x), dtype=np.float32)
    w = np.ascontiguousarray(np.asarray(conv_weight), dtype=np.float32)
    c = np.ascontiguousarray(np.asarray(centroids), dtype=np.float32)
    xs = x.reshape(32, 4, P, S)
    wT = np.ascontiguousarray(w.T).reshape(4, P, K)
    eye = np.eye(P, dtype=np.float32)
    in_maps = [{"X": np.ascontiguousarray(xs[NIMG * i:NIMG * (i + 1)]),
                "WT": wT, "EYE": eye, "CENT": c} for i in range(NCORES)]
    return in_maps


def _run(x, conv_weight, centroids, trace=False):
    from concourse import bass_utils
    nc = _get_nc()
    in_maps = _prep(x, conv_weight, centroids)
    res = bass_utils.run_bass_kernel_spmd(nc, in_maps,
                                          core_ids=list(range(NCORES)),
                                          trace=trace)
    out = np.concatenate(
        [np.asarray(res.results[i]["OUT"]).reshape(NIMG, K * D)
         for i in range(NCORES)], axis=0)
    return out, getattr(res, "exec_time_ns", None)


def kernel(x, conv_weight, centroids):
    out, _ = _run(x, conv_weight, centroids, trace=False)
    return out


# revision 15
# speedup vs baseline: 35014.4138x; 1.5116x over previous
import sys

for _p in ("/opt/trn_rl_repo", "/opt/trn_rl_repo/concourse"):
    if _p not in sys.path:
        sys.path.insert(0, _p)

import numpy as np
import concourse.bass as bass
import concourse.bacc as bacc
import concourse.mybir as mybir
import concourse.tile as tile

P = 128
D = 512
S = 1600
K = 64
NIMG = 4          # images per core
NCORES = 8
NCHUNK = 13       # 12*128 + 64 = 1600
FR = mybir.dt.float32r
F32 = mybir.dt.float32
AF = mybir.ActivationFunctionType
OP = mybir.AluOpType

PIECES = ((0, 256), (256, 768), (768, 1600))  # image-0 load split
AGG_DELAY = 3


def build():
    nc = bacc.Bacc("TRN2", target_bir_lowering=False, debug=False,
                   enable_asserts=True, num_devices=NCORES)
    # host-relaid layouts: X [n, p, g, s], WT [p, g, k]
    X_d = nc.dram_tensor("X", [NIMG, P, 4, S], F32, kind="ExternalInput").ap()
    WT_d = nc.dram_tensor("WT", [P, 4, K], F32, kind="ExternalInput").ap()
    EYE_d = nc.dram_tensor("EYE", [P, P], F32, kind="ExternalInput").ap()
    CENT_d = nc.dram_tensor("CENT", [K, D], F32, kind="ExternalInput").ap()
    OUT_d = nc.dram_tensor("OUT", [NIMG, K, D], F32, kind="ExternalOutput").ap()

    with tile.TileContext(nc) as tc:
        with tc.tile_pool(name="const", bufs=1) as cpool, \
             tc.tile_pool(name="x0", bufs=3) as x0pool, \
             tc.tile_pool(name="xn", bufs=3) as xpool, \
             tc.tile_pool(name="xts", bufs=5) as xtpool, \
             tc.tile_pool(name="exp", bufs=3) as epool, \
             tc.tile_pool(name="e2", bufs=5) as e2pool, \
             tc.tile_pool(name="cols", bufs=5) as colpool, \
             tc.tile_pool(name="sqscr", bufs=1) as sqpool, \
             tc.tile_pool(name="fc", bufs=2) as fcpool, \
             tc.tile_pool(name="nv", bufs=2) as nvpool, \
             tc.tile_pool(name="sq64", bufs=2) as sq64pool, \
             tc.tile_pool(name="outp", bufs=2) as outpool, \
             tc.tile_pool(name="ps_xT", bufs=4, space=bass.MemorySpace.PSUM) as ps_xT, \
             tc.tile_pool(name="ps_lg", bufs=2, space=bass.MemorySpace.PSUM) as ps_lg, \
             tc.tile_pool(name="ps_A", bufs=1, space=bass.MemorySpace.PSUM) as ps_A, \
             tc.tile_pool(name="ps_B", bufs=1, space=bass.MemorySpace.PSUM) as ps_B:

            # Preload act-func table set 6 (natural_log_exp_and_others):
            # contains Square+Ln+Exp+Copy, so the fixpoint table-load pass
            # sees every activation satisfied and inserts no per-chunk reloads.
            nc.scalar.add_instruction(mybir.InstLoadActFuncSet(act_func_set_id=6))
            ident = cpool.tile([P, P], FR)
            wt = cpool.tile([P, 4, K], FR)
            cent = cpool.tile([K, D], F32)
            nc.sync.dma_start(ident[:], EYE_d[:, :].bitcast(FR))
            nc.sync.dma_start(wt[:], WT_d[:, :, :].bitcast(FR))
            sqscr = sqpool.tile([P, D], F32)

            # per-image x slices: image 0 in pieces (early compute start)
            xslices = [[] for _ in range(NIMG)]
            for (c0, c1) in PIECES:
                t = x0pool.tile([P, 4, c1 - c0], FR)
                nc.sync.dma_start(t[:], X_d[0, :, :, c0:c1].bitcast(FR))
                xslices[0].append((t, c0, c1))
            nc.sync.dma_start(cent[:], CENT_d[:, :])

            def load_image(n):
                pieces = ((0, 768), (768, S)) if n == 1 else ((0, S),)
                for (c0, c1) in pieces:
                    t = xpool.tile([P, 4, c1 - c0], FR)
                    nc.sync.dma_start(t[:], X_d[n, :, :, c0:c1].bitcast(FR))
                    xslices[n].append((t, c0, c1))

            for n in range(NIMG):
                if n + 1 < NIMG:
                    load_image(n + 1)
                aggA = ps_A.tile([K, 256], F32)
                aggB = ps_B.tile([K, 258], F32)

                def emit_agg(pend, j):
                    e2, xts, w = pend
                    nc.tensor.matmul(aggA[:, :], e2[0:w, :], xts[0:w, 0:256],
                                     start=(j == 0), stop=(j == NCHUNK - 1))
                    nc.tensor.matmul(aggB[:, :], e2[0:w, :], xts[0:w, 256:514],
                                     start=(j == 0), stop=(j == NCHUNK - 1))

                pending = []
                for j in range(NCHUNK):
                    w = 64 if j == NCHUNK - 1 else 128
                    s0 = j * 128
                    xn, c0, _ = next(sl for sl in xslices[n]
                                     if sl[1] <= s0 < sl[2])
                    l0 = s0 - c0
                    xTp = ps_xT.tile([P, D], F32)
                    lgp = ps_lg.tile([P, K], F32)
                    cols = colpool.tile([P, 8], F32)
                    for g in range(4):
                        nc.tensor.transpose(
                            xTp[0:w, g * 128:(g + 1) * 128].bitcast(FR),
                            xn[:, g, l0:l0 + w], ident[:, :])
                    for g in range(4):
                        nc.tensor.matmul(lgp[0:w, :], xn[:, g, l0:l0 + w],
                                         wt[:, g, :], start=(g == 0), stop=(g == 3))
                    xts = xtpool.tile([P, 514], FR)
                    # raw xT evac (no chain deps -> agg rhs ready early)
                    nc.vector.tensor_scalar(out=xts[0:w, 0:512], in0=xTp[0:w, :],
                                            scalar1=1.0, scalar2=None, op0=OP.mult)
                    # ss = sum_d x^2 per pixel (Pool); inv = exp(-0.5*ln(ss))
                    nc.scalar.activation(out=sqscr[0:w, :], in_=xTp[0:w, :],
                                         func=AF.Square,
                                         accum_out=cols[0:w, 0:1])
                    nc.scalar.activation(out=cols[0:w, 1:2], in_=cols[0:w, 0:1],
                                         func=AF.Ln)
                    nc.scalar.activation(out=cols[0:w, 2:3], in_=cols[0:w, 1:2],
                                         func=AF.Exp, scale=-0.5)
                    # col 512 = ss*inv = ||x|| (gives a_sum); col 513 junk
                    nc.vector.tensor_scalar(out=xts[0:w, 512:514],
                                            in0=cols[0:w, 0:2],
                                            scalar1=cols[0:w, 2:3], scalar2=None,
                                            op0=OP.mult)
                    # e = exp(logits * inv); sumexp
                    expt = epool.tile([P, K], FR)
                    nc.scalar.activation(out=expt[0:w, :], in_=lgp[0:w, :],
                                         func=AF.Exp, scale=cols[0:w, 2:3],
                                         accum_out=cols[0:w, 3:4])
                    nc.vector.reciprocal(cols[0:w, 4:5], cols[0:w, 3:4])  # rse
                    e2 = e2pool.tile([P, K], FR)
                    nc.vector.tensor_scalar(out=e2[0:w, :], in0=expt[0:w, :],
                                            scalar1=cols[0:w, 2:3],
                                            scalar2=cols[0:w, 4:5],
                                            op0=OP.mult, op1=OP.mult)
                    pending.append((e2, xts, w))
                    if len(pending) > AGG_DELAY:
                        emit_agg(pending.pop(0), j - AGG_DELAY)
                for i, pend in enumerate(pending):
                    emit_agg(pend, NCHUNK - len(pending) + i)

                # finale: nv = asum*cent - agg = -vlad; out = nv * (-1/(8*||vlad_k||))
                fc = fcpool.tile([K, 8], F32)
                nv = nvpool.tile([K, D], F32)
                sq64 = sq64pool.tile([K, D], F32)
                nc.scalar.copy(fc[:, 0:1], aggB[:, 256:257])  # a_sum
                nc.vector.scalar_tensor_tensor(out=nv[:, 0:256], in0=cent[:, 0:256],
                                               scalar=fc[:, 0:1], in1=aggA[:, :],
                                               op0=OP.mult, op1=OP.subtract)
                nc.vector.scalar_tensor_tensor(out=nv[:, 256:512], in0=cent[:, 256:512],
                                               scalar=fc[:, 0:1], in1=aggB[:, 0:256],
                                               op0=OP.mult, op1=OP.subtract)
                nc.scalar.activation(out=sq64[:, :], in_=nv[:, :], func=AF.Square,
                                     accum_out=fc[:, 1:2])
                nc.scalar.activation(out=fc[:, 2:3], in_=fc[:, 1:2], func=AF.Ln)
                nc.scalar.activation(out=fc[:, 3:4], in_=fc[:, 2:3], func=AF.Exp,
                                     scale=-0.5)  # 1/sqrt(rowss)
                nc.vector.tensor_scalar(out=fc[:, 4:5], in0=fc[:, 3:4],
                                        scalar1=-0.125, scalar2=None, op0=OP.mult)
                ot = outpool.tile([K, D], F32)
                nc.vector.tensor_scalar(out=ot[:, :], in0=nv[:, :],
                                        scalar1=fc[:, 4:5], scalar2=None, op0=OP.mult)
                nc.sync.dma_start(OUT_d[n, :, :], ot[:, :])
    nc.compile()
    return nc


_NC = None


def _get_nc():
    global _NC
    if _NC is None:
        _NC = build()
    return _NC


def _prep(x, conv_weight, centroids):
    x = np.ascontiguousarray(np.asarray(x), dtype=np.float32)
    w = np.ascontiguousarray(np.asarray(conv_weight), dtype=np.float32)
    c = np.ascontiguousarray(np.asarray(centroids), dtype=np.float32)
    # [32, 4g, 128p, 1600s] -> [32, 128p, 4g, 1600s]
    xs = np.ascontiguousarray(x.reshape(32, 4, P, S).transpose(0, 2, 1, 3))
    # w [64k, 512d] -> [128p, 4g, 64k]
    wT3 = np.ascontiguousarray(w.reshape(K, 4, P).transpose(2, 1, 0))
    eye = np.eye(P, dtype=np.float32)
    in_maps = [{"X": np.ascontiguousarray(xs[NIMG * i:NIMG * (i + 1)]),
                "WT": wT3, "EYE": eye, "CENT": c} for i in range(NCORES)]
    return in_maps


def _run(x, conv_weight, centroids, trace=False):
    from concourse import bass_utils
    nc = _get_nc()
    in_maps = _prep(x, conv_weight, centroids)
    res = bass_utils.run_bass_kernel_spmd(nc, in_maps,
                                          core_ids=list(range(NCORES)),
                                          trace=trace)
    out = np.concatenate(
        [np.asarray(res.results[i]["OUT"]).reshape(NIMG, K * D)
         for i in range(NCORES)], axis=0)
    return out, getattr(res, "exec_time_ns", None)


def kernel(x, conv_weight, centroids):
    out, _ = _run(x, conv_weight, centroids, trace=False)
    return out
